# revision 86
# baseline (speedup 1.0000x reference)
"""KMeans predict (argmin_k ||x - c_k||^2) on 8 TRN2 NeuronCores.

Data-parallel: x [131072, 768] sharded along N across 8 cores (16384 rows
each), centroid table [1024, 768] replicated. Per core, per 128-token tile:
  scores[n, k] = 2*x.c_k - ||c_k||^2   (argmax == argmin of reference)
via f32r matmuls accumulating in PSUM; argmax via DVE max8/max_index.

Key structure (each worth measurable ns on the TimelineSim critical path):
  - x and the centroid table are DMA'd directly into f32r tiles (no ACT
    rounding copies), removing the serialized staging lead-in entirely.
  - the -||c||^2 bias is PRE-WRITTEN into each PSUM tile by ACT and the
    matmuls accumulate on top (start=False), so the per-tile post-matmul
    chain is just the ACT copy-out + DVE argmax. The first PRELUDE tiles
    use start=True + a GPSIMD bias add (the bias row is still in flight
    when they run); the last 8 tiles skip the copy-out and argmax straight
    from PSUM, which shortens the end-of-kernel cascade.
  - table preload fans out across the ACT(HWDGE)/SP(HWDGE)/Pool(SWDGE)
    queues; the bias goes up as a single [1,K] row (tiny DMA) and is
    partition-broadcast on the otherwise-idle GPSIMD. DCORDER matches the
    transfer-arrival order of the serial DMA FIFO.
  - dummy matmuls on a memset tile keep the PE p-state ramping while the
    table streams in.
  - index columns accumulate per 64-tile half; each half is PE-transposed
    and stored as one contiguous DMA. Half A is emitted 3 tiles late so
    the in-order PE never stalls on the DVE chain; only half B's wrap-up
    (DVE argmax + transpose + store) sits on the final critical path.
    (NOTE: a [128,1]-wide read of the last index column right after the
    DVE writes it - via narrow transpose or partition-strided DMA -
    produced partially-stale data on real HW; the full-width half-B
    transpose is the validated safe shape.)

Host-side layout prep (not on the device clock): x pre-transposed into
tile-contiguous [d, n] blocks, centroids into [d, k] blocks as 2*c, and
the -||c||^2 row precomputed.
"""

import sys

sys.path.insert(0, "/opt/trn_rl_repo")

import numpy as np

N, D, K = 131072, 768, 1024
NCORES = 8
NSH = N // NCORES  # 16384 tokens per core
T = NSH // 128     # 128 token-tiles per core
DC = D // 128      # 6 contraction chunks
KHW = 512          # k half-width (one PSUM bank of fp32)
KH = K // KHW      # 2
TH = T // 2        # half of the token tiles (output store granularity)

_nc_cache = []


def _build():
    from concourse import bacc, tile, mybir, masks

    f32 = mybir.dt.float32
    f32r = mybir.dt.float32r
    i32 = mybir.dt.int32
    u32 = mybir.dt.uint32

    nc = bacc.Bacc("TRN2", target_bir_lowering=False, debug=False)
    # xt[t, dlow, dc, n] = x[t*128 + n, dc*128 + dlow]
    xt_d = nc.dram_tensor("xt", [T, 128, DC, 128], f32r, kind="ExternalInput").ap()
    # ct2[dlow, dc, k] = 2 * centroids[k, dc*128 + dlow]
    ct2_d = nc.dram_tensor("ct2", [128, DC, K], f32r, kind="ExternalInput").ap()
    # csqr[0, k] = -||c_k||^2 (single row; broadcast on-device)
    csqr_d = nc.dram_tensor("csqr", [1, K], f32, kind="ExternalInput").ap()
    out = nc.dram_tensor("out", [NSH], i32, kind="ExternalOutput").ap()
    out2d = out.rearrange("(t p) -> t p", p=128)

    # tile 0 accumulates chunks in their arrival order across the three
    # preload queues (it runs during the table stream); later tiles have the
    # full table and use natural order, which lets tile 1 start on the first
    # half of its split x load
    DCORDER_CHASE = [4, 1, 0, 3, 2, 5]
    DCORDER_STEADY = [0, 1, 2, 3, 4, 5]

    with tile.TileContext(nc) as tc:
        with tc.tile_pool(name="const", bufs=1) as constp:
            ident = constp.tile([128, 128], f32)
            ct2 = constp.tile([128, DC, K], f32r)
            csqr = constp.tile([1, K], f32)
            csqb = constp.tile([128, K], f32)
            # preload fan-out across the ACT (HWDGE), Pool (SWDGE) and SP
            # queues; dc4 rides SP ahead of the x-tile stream. The identity
            # (only needed ~170us in, for the store transposes) is built
            # after the Pool queue's DMA triggers so dc1's descriptor
            # generation starts immediately.
            nc.gpsimd.dma_start(ct2[:, 1], ct2_d[:, 1])
            nc.sync.dma_start(ct2[:, 4], ct2_d[:, 4])
            nc.scalar.dma_start(ct2[:, 0], ct2_d[:, 0])
            nc.gpsimd.dma_start(ct2[:, 3], ct2_d[:, 3])
            nc.gpsimd.dma_start(ct2[:, 5], ct2_d[:, 5])
            # dc2 is issued later (inside the loop at t==2): it rides the
            # FIFO behind tile 1's x-pieces, and tiles 0/1's dc2 matmuls are
            # deferred to match
            # the bias row is tiny: DMA one partition, broadcast on the
            # otherwise-idle GPSIMD. Trailing the table is fine: csqb-ready
            # stays well under every consumer's slot.
            nc.scalar.dma_start(csqr[:], csqr_d[:])
            nc.gpsimd.partition_broadcast(csqb[:], csqr[:])
            masks.make_identity(nc, ident[:])

            # ---- main loop over token tiles ----
            with tc.tile_pool(name="xin", bufs=3) as xinp, \
                 tc.tile_pool(name="mainps", bufs=3, space="PSUM") as psp, \
                 tc.tile_pool(name="finps", bufs=1, space="PSUM") as finp, \
                 tc.tile_pool(name="sc0p", bufs=3) as sc0p, \
                 tc.tile_pool(name="scp", bufs=3) as scp, \
                 tc.tile_pool(name="idxcol", bufs=1) as idxp, \
                 tc.tile_pool(name="oip", bufs=2) as oip, \
                 tc.tile_pool(name="small", bufs=3) as smallp:
                # one index-column tile per output half: the PE transpose of
                # half h must not alias the still-filling other half
                fcol_a = idxp.tile([128, TH], f32, tag="fcol_a")
                fcol_b = idxp.tile([128, TH], f32, tag="fcol_b")
                fcols = [fcol_a, fcol_b]

                # warmup: dummy matmuls keep the PE p-state ramping while the
                # centroid-table DMAs are still in flight. The DVE memsets a
                # small operand tile immediately so warmups start at ~0.5us
                # (make_identity on Pool takes ~2us).
                warm_in = constp.tile([128, 128], f32)
                nc.vector.memset(warm_in[:], 0.0)
                warm_ps = psp.tile([128, K], f32, tag="scps")
                for w in range(14):
                    nc.tensor.matmul(
                        warm_ps[:, 0:128], warm_in[:], warm_in[:],
                        start=True, stop=True,
                    )

                # Tiles 0..PRELUDE-1 run the classic start=True path with the
                # bias added by the (otherwise idle) GPSIMD from PSUM — csqb
                # is still in flight when their matmuls begin. For later
                # tiles the -||c||^2 bias is PRE-WRITTEN into PSUM by ACT and
                # the matmuls accumulate on top (start=False), so the
                # per-tile post-matmul chain is just ACT copy-out + DVE
                # argmax.
                PRELUDE = 3
                # the dc=2 chunk is the last to arrive: tiles 0/1 emit their
                # other 10 matmuls first (groups left open) and their dc2
                # pairs are deferred until t==2, so the PE chews tile 1's
                # work while dc2 is still streaming
                DEFER_DC = 2
                sc_ps_next = None
                deferred = []

                def _prelude_post(ps, col):
                    sc_ = scp.tile([128, K], f32, tag="sc")
                    sc0_ = sc0p.tile([128, K], f32, tag="sc0")
                    nc.scalar.copy(sc0_[:], ps[:])
                    nc.gpsimd.tensor_add(sc_[:], sc0_[:], csqb[:])
                    mx_ = smallp.tile([128, 8], f32, tag="mx")
                    mi_ = smallp.tile([128, 8], u32, tag="mi")
                    nc.vector.max(mx_[:], sc_[:])
                    nc.vector.max_index(mi_[:], mx_[:], sc_[:])
                    nc.vector.tensor_copy(
                        fcols[0][:, col:col + 1], mi_[:, 0:1])

                for t in range(T):
                    xin = xinp.tile([128, DC, 128], f32r, tag="xin")
                    # tiles 1-2 load via the ACT queue: their transfers then
                    # enter the serial DMA FIFO behind the early table chunks
                    if t == 1:
                        # three pieces so tile 1's matmuls chase their own
                        # x-stream with no single big wait
                        nc.scalar.dma_start(xin[:, 0:2], xt_d[t][:, 0:2])
                        nc.scalar.dma_start(xin[:, 2:4], xt_d[t][:, 2:4])
                        nc.scalar.dma_start(xin[:, 4:DC], xt_d[t][:, 4:DC])
                    elif t == 2:
                        # the deferred table chunk goes out now (after tile
                        # 1's x-pieces in the FIFO), then tile 2's x
                        nc.scalar.dma_start(ct2[:, DEFER_DC],
                                            ct2_d[:, DEFER_DC])
                        nc.scalar.dma_start(xin[:], xt_d[t])
                    else:
                        nc.sync.dma_start(xin[:], xt_d[t])

                    if t == 2:
                        # flush the deferred dc2 pairs (closing tiles 0/1's
                        # accumulation groups), then their post-processing
                        for tp, ps, xi in deferred:
                            for kh in range(KH):
                                ksl = slice(kh * KHW, (kh + 1) * KHW)
                                nc.tensor.matmul(
                                    ps[:, ksl],
                                    xi[:, DEFER_DC, :],
                                    ct2[:, DEFER_DC, ksl],
                                    start=False,
                                    stop=True,
                                )
                        for tp, ps, xi in deferred:
                            _prelude_post(ps, tp)

                    prelude = t < PRELUDE
                    if prelude:
                        sc_ps = psp.tile([128, K], f32, tag="scps")
                    else:
                        sc_ps = sc_ps_next
                    # chase tiles interleave their two kh accumulation groups
                    # chunk-major so late-arriving chunks leave at most one
                    # pair of matmuls serial in the in-order stream
                    if t == 0:
                        order = [dc for dc in DCORDER_CHASE if dc != DEFER_DC]
                        mmseq = [(kh, dc, j == 0, False)
                                 for j, dc in enumerate(order)
                                 for kh in range(KH)]
                    elif t == 1:
                        order = [dc for dc in DCORDER_STEADY if dc != DEFER_DC]
                        mmseq = [(kh, dc, j == 0, False)
                                 for j, dc in enumerate(order)
                                 for kh in range(KH)]
                    else:
                        mmseq = [(kh, dc, j == 0, j == DC - 1)
                                 for kh in range(KH)
                                 for j, dc in enumerate(DCORDER_STEADY)]
                    for kh, dc, first, last in mmseq:
                        ksl = slice(kh * KHW, (kh + 1) * KHW)
                        nc.tensor.matmul(
                            sc_ps[:, ksl],
                            xin[:, dc, :],
                            ct2[:, dc, ksl],
                            start=(first and prelude),
                            stop=last,
                        )
                    if t < 2:
                        deferred.append((t, sc_ps, xin))
                        continue
                    if PRELUDE <= t + 1 < T:
                        sc_ps_next = psp.tile([128, K], f32, tag="scps")
                        nc.scalar.copy(sc_ps_next[:], csqb[:])
                    if prelude:
                        # GPSIMD cannot access PSUM: ACT copies out, then the
                        # (otherwise idle) GPSIMD adds the bias in SBUF
                        sc = scp.tile([128, K], f32, tag="sc")
                        sc0 = sc0p.tile([128, K], f32, tag="sc0")
                        nc.scalar.copy(sc0[:], sc_ps[:])
                        nc.gpsimd.tensor_add(sc[:], sc0[:], csqb[:])
                    elif t < T - 8:
                        sc = scp.tile([128, K], f32, tag="sc")
                        nc.scalar.copy(sc[:], sc_ps[:])
                    else:
                        # final tiles: skip the copy-out; DVE argmaxes
                        # straight from PSUM (the +250ns/op PSUM-access cost
                        # beats the ~1.3us ACT-copy latency on the final
                        # cascade)
                        sc = sc_ps
                    mx = smallp.tile([128, 8], f32, tag="mx")
                    mi = smallp.tile([128, 8], u32, tag="mi")
                    nc.vector.max(mx[:], sc[:])
                    nc.vector.max_index(mi[:], mx[:], sc[:])
                    nc.vector.tensor_copy(
                        fcols[t // TH][:, t % TH:t % TH + 1], mi[:, 0:1])

                    # stores: transpose the finished half's index columns
                    # [token_in_tile, tile] -> [tile, token_in_tile] and
                    # store. Half A is emitted a few tiles late so the
                    # in-order PE doesn't stall on the DVE chain.
                    if t == TH - 1 + 3 or t == T - 1:
                        h = 0 if t < T - 1 else 1
                        hsl = slice(h * TH, (h + 1) * TH)
                        ftps = finp.tile([TH, 128], f32, tag=f"ftps{h}")
                        nc.tensor.transpose(ftps[:, :], fcols[h][:], ident[:])
                        oi = oip.tile([TH, 128], i32, tag=f"oi{h}")
                        if h == 0:
                            nc.scalar.copy(oi[:], ftps[:, :])
                        else:
                            # half B's convert rides DVE (free right after its
                            # last index-column copy): shorter final cascade
                            # than hopping to ACT
                            nc.vector.tensor_copy(oi[:], ftps[:, :])
                        nc.sync.dma_start(out2d[hsl], oi[:])

    nc.compile()
    return nc


def _get_nc():
    if not _nc_cache:
        _nc_cache.append(_build())
    return _nc_cache[0]


def _prep(x, centroids):
    x = np.ascontiguousarray(np.asarray(x), dtype=np.float32)
    c = np.ascontiguousarray(np.asarray(centroids), dtype=np.float32)
    ct2 = np.ascontiguousarray((2.0 * c).reshape(K, DC, 128).transpose(2, 1, 0))
    csqr = np.ascontiguousarray(
        -(c * c).sum(-1, dtype=np.float32).reshape(1, K)
    )
    in_maps = []
    for i in range(NCORES):
        sh = x[i * NSH:(i + 1) * NSH]
        # [t, n, dc, dlow] -> [t, dlow, dc, n]
        xt = np.ascontiguousarray(
            sh.reshape(T, 128, DC, 128).transpose(0, 3, 2, 1)
        )
        in_maps.append({"xt": xt, "ct2": ct2, "csqr": csqr})
    return in_maps


def kernel(x, centroids):
    from concourse import bass_utils

    nc = _get_nc()
    in_maps = _prep(x, centroids)
    res = bass_utils.run_bass_kernel_spmd(nc, in_maps, core_ids=list(range(NCORES)))
    return np.concatenate([res.results[i]["out"] for i in range(NCORES)])



# revision 94
# speedup vs baseline: 1.0011x; 1.0011x over previous
"""KMeans predict (argmin_k ||x - c_k||^2) on 8 TRN2 NeuronCores.

Data-parallel: x [131072, 768] sharded along N across 8 cores (16384 rows
each), centroid table [1024, 768] replicated. Per core, per 128-token tile:
  scores[n, k] = 2*x.c_k - ||c_k||^2   (argmax == argmin of reference)
via f32r matmuls accumulating in PSUM; argmax via DVE max8/max_index.

Key structure (each worth measurable ns on the TimelineSim critical path):
  - x and the centroid table are DMA'd directly into f32r tiles (no ACT
    rounding copies), removing the serialized staging lead-in entirely.
  - the -||c||^2 bias is PRE-WRITTEN into each PSUM tile by ACT and the
    matmuls accumulate on top (start=False), so the per-tile post-matmul
    chain is just the ACT copy-out + DVE argmax. The first PRELUDE tiles
    use start=True + a GPSIMD bias add (the bias row is still in flight
    when they run); the last 8 tiles skip the copy-out and argmax straight
    from PSUM, which shortens the end-of-kernel cascade.
  - table preload fans out across the ACT(HWDGE)/SP(HWDGE)/Pool(SWDGE)
    queues; the bias goes up as a single [1,K] row (tiny DMA) and is
    partition-broadcast on the otherwise-idle GPSIMD. DCORDER matches the
    transfer-arrival order of the serial DMA FIFO.
  - dummy matmuls on a memset tile keep the PE p-state ramping while the
    table streams in.
  - index columns accumulate per 64-tile half; each half is PE-transposed
    and stored as one contiguous DMA. Half A is emitted 3 tiles late so
    the in-order PE never stalls on the DVE chain; only half B's wrap-up
    (DVE argmax + transpose + store) sits on the final critical path.
    (NOTE: a [128,1]-wide read of the last index column right after the
    DVE writes it - via narrow transpose or partition-strided DMA -
    produced partially-stale data on real HW; the full-width half-B
    transpose is the validated safe shape.)

Host-side layout prep (not on the device clock): x pre-transposed into
tile-contiguous [d, n] blocks, centroids into [d, k] blocks as 2*c, and
the -||c||^2 row precomputed.
"""

import sys

sys.path.insert(0, "/opt/trn_rl_repo")

import numpy as np

N, D, K = 131072, 768, 1024
NCORES = 8
NSH = N // NCORES  # 16384 tokens per core
T = NSH // 128     # 128 token-tiles per core
DC = D // 128      # 6 contraction chunks
KHW = 512          # k half-width (one PSUM bank of fp32)
KH = K // KHW      # 2
TH = T // 2        # half of the token tiles (output store granularity)

_nc_cache = []


def _build():
    from concourse import bacc, tile, mybir, masks

    f32 = mybir.dt.float32
    f32r = mybir.dt.float32r
    i32 = mybir.dt.int32
    u32 = mybir.dt.uint32

    nc = bacc.Bacc("TRN2", target_bir_lowering=False, debug=False)
    # xt[t, dlow, dc, n] = x[t*128 + n, dc*128 + dlow]
    xt_d = nc.dram_tensor("xt", [T, 128, DC, 128], f32r, kind="ExternalInput").ap()
    # ct2[dlow, dc, k] = 2 * centroids[k, dc*128 + dlow]
    ct2_d = nc.dram_tensor("ct2", [128, DC, K], f32r, kind="ExternalInput").ap()
    # csqr[0, k] = -||c_k||^2 (single row; broadcast on-device)
    csqr_d = nc.dram_tensor("csqr", [1, K], f32, kind="ExternalInput").ap()
    out = nc.dram_tensor("out", [NSH], i32, kind="ExternalOutput").ap()
    out2d = out.rearrange("(t p) -> t p", p=128)

    # tile 0 accumulates chunks in their arrival order across the three
    # preload queues (it runs during the table stream); later tiles have the
    # full table and use natural order, which lets tile 1 start on the first
    # half of its split x load
    DCORDER_CHASE = [4, 1, 0, 3, 2, 5]
    DCORDER_STEADY = [0, 1, 2, 3, 4, 5]

    with tile.TileContext(nc) as tc:
        with tc.tile_pool(name="const", bufs=1) as constp:
            ident = constp.tile([128, 128], f32)
            ct2 = constp.tile([128, DC, K], f32r)
            csqr = constp.tile([1, K], f32)
            csqb = constp.tile([128, K], f32)
            # preload fan-out across the ACT (HWDGE), Pool (SWDGE) and SP
            # queues; dc4 rides SP ahead of the x-tile stream. The identity
            # (only needed ~170us in, for the store transposes) is built
            # after the Pool queue's DMA triggers so dc1's descriptor
            # generation starts immediately.
            nc.gpsimd.dma_start(ct2[:, 1], ct2_d[:, 1])
            nc.sync.dma_start(ct2[:, 4], ct2_d[:, 4])
            nc.scalar.dma_start(ct2[:, 0], ct2_d[:, 0])
            nc.gpsimd.dma_start(ct2[:, 3], ct2_d[:, 3])
            nc.gpsimd.dma_start(ct2[:, 5], ct2_d[:, 5])
            # dc2 is issued later (inside the loop at t==2): it rides the
            # FIFO behind tile 1's x-pieces, and tiles 0/1's dc2 matmuls are
            # deferred to match
            # the bias row is tiny: DMA one partition, broadcast on the
            # otherwise-idle GPSIMD. Trailing the table is fine: csqb-ready
            # stays well under every consumer's slot.
            nc.scalar.dma_start(csqr[:], csqr_d[:])
            nc.gpsimd.partition_broadcast(csqb[:], csqr[:])
            masks.make_identity(nc, ident[:])

            # ---- main loop over token tiles ----
            with tc.tile_pool(name="xin", bufs=3) as xinp, \
                 tc.tile_pool(name="mainps", bufs=3, space="PSUM") as psp, \
                 tc.tile_pool(name="finps", bufs=1, space="PSUM") as finp, \
                 tc.tile_pool(name="sc0p", bufs=3) as sc0p, \
                 tc.tile_pool(name="scp", bufs=3) as scp, \
                 tc.tile_pool(name="idxcol", bufs=1) as idxp, \
                 tc.tile_pool(name="oip", bufs=2) as oip, \
                 tc.tile_pool(name="small", bufs=3) as smallp:
                # one index-column tile per output half: the PE transpose of
                # half h must not alias the still-filling other half
                fcol_a = idxp.tile([128, TH], f32, tag="fcol_a")
                fcol_b = idxp.tile([128, TH], f32, tag="fcol_b")
                fcols = [fcol_a, fcol_b]

                # warmup: dummy matmuls keep the PE p-state ramping while the
                # centroid-table DMAs are still in flight. The DVE memsets a
                # small operand tile immediately so warmups start at ~0.5us
                # (make_identity on Pool takes ~2us).
                warm_in = constp.tile([128, 128], f32)
                nc.vector.memset(warm_in[:], 0.0)
                warm_ps = psp.tile([128, K], f32, tag="scps")
                for w in range(18):
                    nc.tensor.matmul(
                        warm_ps[:, 0:128], warm_in[:], warm_in[:],
                        start=True, stop=True,
                    )

                # Tiles 0..PRELUDE-1 run the classic start=True path with the
                # bias added by the (otherwise idle) GPSIMD from PSUM — csqb
                # is still in flight when their matmuls begin. For later
                # tiles the -||c||^2 bias is PRE-WRITTEN into PSUM by ACT and
                # the matmuls accumulate on top (start=False), so the
                # per-tile post-matmul chain is just ACT copy-out + DVE
                # argmax.
                PRELUDE = 3
                # the dc=2 chunk is the last to arrive: tiles 0/1 emit their
                # other 10 matmuls first (groups left open) and their dc2
                # pairs are deferred until t==2, so the PE chews tile 1's
                # work while dc2 is still streaming
                DEFER_DC = 2
                sc_ps_next = None
                deferred = []

                def _prelude_post(ps, col):
                    sc_ = scp.tile([128, K], f32, tag="sc")
                    sc0_ = sc0p.tile([128, K], f32, tag="sc0")
                    nc.scalar.copy(sc0_[:], ps[:])
                    nc.gpsimd.tensor_add(sc_[:], sc0_[:], csqb[:])
                    mx_ = smallp.tile([128, 8], f32, tag="mx")
                    mi_ = smallp.tile([128, 8], u32, tag="mi")
                    nc.vector.max(mx_[:], sc_[:])
                    nc.vector.max_index(mi_[:], mx_[:], sc_[:])
                    nc.vector.tensor_copy(
                        fcols[0][:, col:col + 1], mi_[:, 0:1])

                for t in range(T):
                    xin = xinp.tile([128, DC, 128], f32r, tag="xin")
                    # tiles 1-2 load via the ACT queue: their transfers then
                    # enter the serial DMA FIFO behind the early table chunks
                    if t == 1:
                        # three pieces so tile 1's matmuls chase their own
                        # x-stream with no single big wait
                        nc.scalar.dma_start(xin[:, 0:2], xt_d[t][:, 0:2])
                        nc.scalar.dma_start(xin[:, 2:4], xt_d[t][:, 2:4])
                        nc.scalar.dma_start(xin[:, 4:DC], xt_d[t][:, 4:DC])
                    elif t == 2:
                        # the deferred table chunk goes out now (after tile
                        # 1's x-pieces in the FIFO), then tile 2's x
                        nc.scalar.dma_start(ct2[:, DEFER_DC],
                                            ct2_d[:, DEFER_DC])
                        nc.scalar.dma_start(xin[:], xt_d[t])
                    else:
                        nc.sync.dma_start(xin[:], xt_d[t])

                    if t == 2:
                        # flush the deferred dc2 pairs (closing tiles 0/1's
                        # accumulation groups), then their post-processing
                        for tp, ps, xi in deferred:
                            for kh in range(KH):
                                ksl = slice(kh * KHW, (kh + 1) * KHW)
                                nc.tensor.matmul(
                                    ps[:, ksl],
                                    xi[:, DEFER_DC, :],
                                    ct2[:, DEFER_DC, ksl],
                                    start=False,
                                    stop=True,
                                )
                        for tp, ps, xi in deferred:
                            _prelude_post(ps, tp)

                    prelude = t < PRELUDE
                    if prelude:
                        sc_ps = psp.tile([128, K], f32, tag="scps")
                    else:
                        sc_ps = sc_ps_next
                    # chase tiles interleave their two kh accumulation groups
                    # chunk-major so late-arriving chunks leave at most one
                    # pair of matmuls serial in the in-order stream
                    if t == 0:
                        order = [dc for dc in DCORDER_CHASE if dc != DEFER_DC]
                        mmseq = [(kh, dc, j == 0, False)
                                 for j, dc in enumerate(order)
                                 for kh in range(KH)]
                    elif t == 1:
                        order = [dc for dc in DCORDER_STEADY if dc != DEFER_DC]
                        mmseq = [(kh, dc, j == 0, False)
                                 for j, dc in enumerate(order)
                                 for kh in range(KH)]
                    else:
                        mmseq = [(kh, dc, j == 0, j == DC - 1)
                                 for kh in range(KH)
                                 for j, dc in enumerate(DCORDER_STEADY)]
                    for kh, dc, first, last in mmseq:
                        ksl = slice(kh * KHW, (kh + 1) * KHW)
                        nc.tensor.matmul(
                            sc_ps[:, ksl],
                            xin[:, dc, :],
                            ct2[:, dc, ksl],
                            start=(first and prelude),
                            stop=last,
                        )
                    if t < 2:
                        deferred.append((t, sc_ps, xin))
                        continue
                    if PRELUDE <= t + 1 < T:
                        sc_ps_next = psp.tile([128, K], f32, tag="scps")
                        nc.scalar.copy(sc_ps_next[:], csqb[:])
                    if prelude:
                        # GPSIMD cannot access PSUM: ACT copies out, then the
                        # (otherwise idle) GPSIMD adds the bias in SBUF
                        sc = scp.tile([128, K], f32, tag="sc")
                        sc0 = sc0p.tile([128, K], f32, tag="sc0")
                        nc.scalar.copy(sc0[:], sc_ps[:])
                        nc.gpsimd.tensor_add(sc[:], sc0[:], csqb[:])
                    elif t < T - 8:
                        sc = scp.tile([128, K], f32, tag="sc")
                        nc.scalar.copy(sc[:], sc_ps[:])
                    else:
                        # final tiles: skip the copy-out; DVE argmaxes
                        # straight from PSUM (the +250ns/op PSUM-access cost
                        # beats the ~1.3us ACT-copy latency on the final
                        # cascade)
                        sc = sc_ps
                    mx = smallp.tile([128, 8], f32, tag="mx")
                    mi = smallp.tile([128, 8], u32, tag="mi")
                    nc.vector.max(mx[:], sc[:])
                    nc.vector.max_index(mi[:], mx[:], sc[:])
                    nc.vector.tensor_copy(
                        fcols[t // TH][:, t % TH:t % TH + 1], mi[:, 0:1])

                    # stores: transpose the finished half's index columns
                    # [token_in_tile, tile] -> [tile, token_in_tile] and
                    # store. Half A is emitted a few tiles late so the
                    # in-order PE doesn't stall on the DVE chain.
                    if t == TH - 1 + 3 or t == T - 1:
                        h = 0 if t < T - 1 else 1
                        hsl = slice(h * TH, (h + 1) * TH)
                        ftps = finp.tile([TH, 128], f32, tag=f"ftps{h}")
                        nc.tensor.transpose(ftps[:, :], fcols[h][:], ident[:])
                        oi = oip.tile([TH, 128], i32, tag=f"oi{h}")
                        if h == 0:
                            nc.scalar.copy(oi[:], ftps[:, :])
                        else:
                            # half B's convert rides DVE (free right after its
                            # last index-column copy): shorter final cascade
                            # than hopping to ACT
                            nc.vector.tensor_copy(oi[:], ftps[:, :])
                        nc.sync.dma_start(out2d[hsl], oi[:])

    nc.compile()
    return nc


def _get_nc():
    if not _nc_cache:
        _nc_cache.append(_build())
    return _nc_cache[0]


def _prep(x, centroids):
    x = np.ascontiguousarray(np.asarray(x), dtype=np.float32)
    c = np.ascontiguousarray(np.asarray(centroids), dtype=np.float32)
    ct2 = np.ascontiguousarray((2.0 * c).reshape(K, DC, 128).transpose(2, 1, 0))
    csqr = np.ascontiguousarray(
        -(c * c).sum(-1, dtype=np.float32).reshape(1, K)
    )
    in_maps = []
    for i in range(NCORES):
        sh = x[i * NSH:(i + 1) * NSH]
        # [t, n, dc, dlow] -> [t, dlow, dc, n]
        xt = np.ascontiguousarray(
            sh.reshape(T, 128, DC, 128).transpose(0, 3, 2, 1)
        )
        in_maps.append({"xt": xt, "ct2": ct2, "csqr": csqr})
    return in_maps


def kernel(x, centroids):
    from concourse import bass_utils

    nc = _get_nc()
    in_maps = _prep(x, centroids)
    res = bass_utils.run_bass_kernel_spmd(nc, in_maps, core_ids=list(range(NCORES)))
    return np.concatenate([res.results[i]["out"] for i in range(NCORES)])



# revision 97
# speedup vs baseline: 1.0012x; 1.0002x over previous
"""KMeans predict (argmin_k ||x - c_k||^2) on 8 TRN2 NeuronCores.

Data-parallel: x [131072, 768] sharded along N across 8 cores (16384 rows
each), centroid table [1024, 768] replicated. Per core, per 128-token tile:
  scores[n, k] = 2*x.c_k - ||c_k||^2   (argmax == argmin of reference)
via f32r matmuls accumulating in PSUM; argmax via DVE max8/max_index.

Key structure (each worth measurable ns on the TimelineSim critical path):
  - x and the centroid table are DMA'd directly into f32r tiles (no ACT
    rounding copies), removing the serialized staging lead-in entirely.
  - the -||c||^2 bias is PRE-WRITTEN into each PSUM tile by ACT and the
    matmuls accumulate on top (start=False), so the per-tile post-matmul
    chain is just the ACT copy-out + DVE argmax. The first PRELUDE tiles
    use start=True + a GPSIMD bias add (the bias row is still in flight
    when they run); the last 8 tiles skip the copy-out and argmax straight
    from PSUM, which shortens the end-of-kernel cascade.
  - table preload fans out across the ACT(HWDGE)/SP(HWDGE)/Pool(SWDGE)
    queues; the bias goes up as a single [1,K] row (tiny DMA) and is
    partition-broadcast on the otherwise-idle GPSIMD. DCORDER matches the
    transfer-arrival order of the serial DMA FIFO.
  - dummy matmuls on a memset tile keep the PE p-state ramping while the
    table streams in.
  - index columns accumulate per 64-tile half; each half is PE-transposed
    and stored as one contiguous DMA. Half A is emitted 3 tiles late so
    the in-order PE never stalls on the DVE chain; only half B's wrap-up
    (DVE argmax + transpose + store) sits on the final critical path.
    (NOTE: a [128,1]-wide read of the last index column right after the
    DVE writes it - via narrow transpose or partition-strided DMA -
    produced partially-stale data on real HW; the full-width half-B
    transpose is the validated safe shape.)

Host-side layout prep (not on the device clock): x pre-transposed into
tile-contiguous [d, n] blocks, centroids into [d, k] blocks as 2*c, and
the -||c||^2 row precomputed.
"""

import sys

sys.path.insert(0, "/opt/trn_rl_repo")

import numpy as np

N, D, K = 131072, 768, 1024
NCORES = 8
NSH = N // NCORES  # 16384 tokens per core
T = NSH // 128     # 128 token-tiles per core
DC = D // 128      # 6 contraction chunks
KHW = 512          # k half-width (one PSUM bank of fp32)
KH = K // KHW      # 2
TH = T // 2        # half of the token tiles (output store granularity)

_nc_cache = []


def _build():
    from concourse import bacc, tile, mybir, masks

    f32 = mybir.dt.float32
    f32r = mybir.dt.float32r
    i32 = mybir.dt.int32
    u32 = mybir.dt.uint32

    nc = bacc.Bacc("TRN2", target_bir_lowering=False, debug=False)
    # xt[t, dlow, dc, n] = x[t*128 + n, dc*128 + dlow]
    xt_d = nc.dram_tensor("xt", [T, 128, DC, 128], f32r, kind="ExternalInput").ap()
    # ct2[dlow, dc, k] = 2 * centroids[k, dc*128 + dlow]
    ct2_d = nc.dram_tensor("ct2", [128, DC, K], f32r, kind="ExternalInput").ap()
    # csqr[0, k] = -||c_k||^2 (single row; broadcast on-device)
    csqr_d = nc.dram_tensor("csqr", [1, K], f32, kind="ExternalInput").ap()
    out = nc.dram_tensor("out", [NSH], i32, kind="ExternalOutput").ap()
    out2d = out.rearrange("(t p) -> t p", p=128)

    # tile 0 accumulates chunks in their arrival order across the three
    # preload queues (it runs during the table stream); later tiles have the
    # full table and use natural order, which lets tile 1 start on the first
    # half of its split x load
    DCORDER_CHASE = [4, 1, 0, 3, 2, 5]
    DCORDER_STEADY = [0, 1, 2, 3, 4, 5]

    with tile.TileContext(nc) as tc:
        with tc.tile_pool(name="const", bufs=1) as constp:
            ident = constp.tile([128, 128], f32)
            ct2 = constp.tile([128, DC, K], f32r)
            csqr = constp.tile([1, K], f32)
            csqb = constp.tile([128, K], f32)
            # preload fan-out across the ACT (HWDGE), Pool (SWDGE) and SP
            # queues; dc4 rides SP ahead of the x-tile stream. The identity
            # (only needed ~170us in, for the store transposes) is built
            # after the Pool queue's DMA triggers so dc1's descriptor
            # generation starts immediately.
            nc.gpsimd.dma_start(ct2[:, 1], ct2_d[:, 1])
            nc.sync.dma_start(ct2[:, 4], ct2_d[:, 4])
            nc.scalar.dma_start(ct2[:, 0], ct2_d[:, 0])
            nc.gpsimd.dma_start(ct2[:, 3], ct2_d[:, 3])
            nc.gpsimd.dma_start(ct2[:, 5], ct2_d[:, 5])
            # dc2 is issued later (inside the loop at t==2): it rides the
            # FIFO behind tile 1's x-pieces, and tiles 0/1's dc2 matmuls are
            # deferred to match
            # the bias row is tiny: DMA one partition, broadcast on the
            # otherwise-idle GPSIMD. Trailing the table is fine: csqb-ready
            # stays well under every consumer's slot.
            nc.scalar.dma_start(csqr[:], csqr_d[:])
            nc.gpsimd.partition_broadcast(csqb[:], csqr[:])
            masks.make_identity(nc, ident[:])

            # ---- main loop over token tiles ----
            with tc.tile_pool(name="xin", bufs=3) as xinp, \
                 tc.tile_pool(name="mainps", bufs=3, space="PSUM") as psp, \
                 tc.tile_pool(name="finps", bufs=1, space="PSUM") as finp, \
                 tc.tile_pool(name="sc0p", bufs=3) as sc0p, \
                 tc.tile_pool(name="scp", bufs=3) as scp, \
                 tc.tile_pool(name="idxcol", bufs=1) as idxp, \
                 tc.tile_pool(name="oip", bufs=2) as oip, \
                 tc.tile_pool(name="small", bufs=3) as smallp:
                # one index-column tile per output half: the PE transpose of
                # half h must not alias the still-filling other half
                fcol_a = idxp.tile([128, TH], f32, tag="fcol_a")
                fcol_b = idxp.tile([128, TH], f32, tag="fcol_b")
                fcols = [fcol_a, fcol_b]

                # warmup: dummy matmuls keep the PE p-state ramping while the
                # centroid-table DMAs are still in flight. The DVE memsets a
                # small operand tile immediately so warmups start at ~0.5us
                # (make_identity on Pool takes ~2us).
                warm_in = constp.tile([128, 128], f32)
                nc.vector.memset(warm_in[:], 0.0)
                warm_ps = psp.tile([128, K], f32, tag="scps")
                for w in range(18):
                    nc.tensor.matmul(
                        warm_ps[:, 0:128], warm_in[:], warm_in[:],
                        start=True, stop=True,
                    )

                # Tiles 0..PRELUDE-1 run the classic start=True path with the
                # bias added by the (otherwise idle) GPSIMD from PSUM — csqb
                # is still in flight when their matmuls begin. For later
                # tiles the -||c||^2 bias is PRE-WRITTEN into PSUM by ACT and
                # the matmuls accumulate on top (start=False), so the
                # per-tile post-matmul chain is just ACT copy-out + DVE
                # argmax.
                PRELUDE = 4
                # the dc=2 chunk is the last to arrive: tiles 0/1 emit their
                # other 10 matmuls first (groups left open) and their dc2
                # pairs are deferred until t==2, so the PE chews tile 1's
                # work while dc2 is still streaming
                DEFER_DC = 2
                sc_ps_next = None
                deferred = []

                def _prelude_post(ps, col):
                    sc_ = scp.tile([128, K], f32, tag="sc")
                    sc0_ = sc0p.tile([128, K], f32, tag="sc0")
                    nc.scalar.copy(sc0_[:], ps[:])
                    nc.gpsimd.tensor_add(sc_[:], sc0_[:], csqb[:])
                    mx_ = smallp.tile([128, 8], f32, tag="mx")
                    mi_ = smallp.tile([128, 8], u32, tag="mi")
                    nc.vector.max(mx_[:], sc_[:])
                    nc.vector.max_index(mi_[:], mx_[:], sc_[:])
                    nc.vector.tensor_copy(
                        fcols[0][:, col:col + 1], mi_[:, 0:1])

                for t in range(T):
                    xin = xinp.tile([128, DC, 128], f32r, tag="xin")
                    # tiles 1-2 load via the ACT queue: their transfers then
                    # enter the serial DMA FIFO behind the early table chunks
                    if t == 1:
                        # three pieces so tile 1's matmuls chase their own
                        # x-stream with no single big wait
                        nc.scalar.dma_start(xin[:, 0:2], xt_d[t][:, 0:2])
                        nc.scalar.dma_start(xin[:, 2:4], xt_d[t][:, 2:4])
                        nc.scalar.dma_start(xin[:, 4:DC], xt_d[t][:, 4:DC])
                    elif t == 2:
                        # the deferred table chunk goes out now (after tile
                        # 1's x-pieces in the FIFO), then tile 2's x
                        nc.scalar.dma_start(ct2[:, DEFER_DC],
                                            ct2_d[:, DEFER_DC])
                        nc.scalar.dma_start(xin[:], xt_d[t])
                    else:
                        nc.sync.dma_start(xin[:], xt_d[t])

                    if t == 2:
                        # flush the deferred dc2 pairs (closing tiles 0/1's
                        # accumulation groups), then their post-processing
                        for tp, ps, xi in deferred:
                            for kh in range(KH):
                                ksl = slice(kh * KHW, (kh + 1) * KHW)
                                nc.tensor.matmul(
                                    ps[:, ksl],
                                    xi[:, DEFER_DC, :],
                                    ct2[:, DEFER_DC, ksl],
                                    start=False,
                                    stop=True,
                                )
                        for tp, ps, xi in deferred:
                            _prelude_post(ps, tp)

                    prelude = t < PRELUDE
                    if prelude:
                        sc_ps = psp.tile([128, K], f32, tag="scps")
                    else:
                        sc_ps = sc_ps_next
                    # chase tiles interleave their two kh accumulation groups
                    # chunk-major so late-arriving chunks leave at most one
                    # pair of matmuls serial in the in-order stream
                    if t == 0:
                        order = [dc for dc in DCORDER_CHASE if dc != DEFER_DC]
                        mmseq = [(kh, dc, j == 0, False)
                                 for j, dc in enumerate(order)
                                 for kh in range(KH)]
                    elif t == 1:
                        order = [dc for dc in DCORDER_STEADY if dc != DEFER_DC]
                        mmseq = [(kh, dc, j == 0, False)
                                 for j, dc in enumerate(order)
                                 for kh in range(KH)]
                    else:
                        mmseq = [(kh, dc, j == 0, j == DC - 1)
                                 for kh in range(KH)
                                 for j, dc in enumerate(DCORDER_STEADY)]
                    for kh, dc, first, last in mmseq:
                        ksl = slice(kh * KHW, (kh + 1) * KHW)
                        nc.tensor.matmul(
                            sc_ps[:, ksl],
                            xin[:, dc, :],
                            ct2[:, dc, ksl],
                            start=(first and prelude),
                            stop=last,
                        )
                    if t < 2:
                        deferred.append((t, sc_ps, xin))
                        continue
                    if PRELUDE <= t + 1 < T:
                        sc_ps_next = psp.tile([128, K], f32, tag="scps")
                        nc.scalar.copy(sc_ps_next[:], csqb[:])
                    if prelude:
                        # GPSIMD cannot access PSUM: ACT copies out, then the
                        # (otherwise idle) GPSIMD adds the bias in SBUF
                        sc = scp.tile([128, K], f32, tag="sc")
                        sc0 = sc0p.tile([128, K], f32, tag="sc0")
                        nc.scalar.copy(sc0[:], sc_ps[:])
                        nc.gpsimd.tensor_add(sc[:], sc0[:], csqb[:])
                    elif t < T - 8:
                        sc = scp.tile([128, K], f32, tag="sc")
                        nc.scalar.copy(sc[:], sc_ps[:])
                    else:
                        # final tiles: skip the copy-out; DVE argmaxes
                        # straight from PSUM (the +250ns/op PSUM-access cost
                        # beats the ~1.3us ACT-copy latency on the final
                        # cascade)
                        sc = sc_ps
                    mx = smallp.tile([128, 8], f32, tag="mx")
                    mi = smallp.tile([128, 8], u32, tag="mi")
                    nc.vector.max(mx[:], sc[:])
                    nc.vector.max_index(mi[:], mx[:], sc[:])
                    nc.vector.tensor_copy(
                        fcols[t // TH][:, t % TH:t % TH + 1], mi[:, 0:1])

                    # stores: transpose the finished half's index columns
                    # [token_in_tile, tile] -> [tile, token_in_tile] and
                    # store. Half A is emitted a few tiles late so the
                    # in-order PE doesn't stall on the DVE chain.
                    if t == TH - 1 + 3 or t == T - 1:
                        h = 0 if t < T - 1 else 1
                        hsl = slice(h * TH, (h + 1) * TH)
                        ftps = finp.tile([TH, 128], f32, tag=f"ftps{h}")
                        nc.tensor.transpose(ftps[:, :], fcols[h][:], ident[:])
                        oi = oip.tile([TH, 128], i32, tag=f"oi{h}")
                        if h == 0:
                            nc.scalar.copy(oi[:], ftps[:, :])
                        else:
                            # half B's convert rides DVE (free right after its
                            # last index-column copy): shorter final cascade
                            # than hopping to ACT
                            nc.vector.tensor_copy(oi[:], ftps[:, :])
                        nc.sync.dma_start(out2d[hsl], oi[:])

    nc.compile()
    return nc


def _get_nc():
    if not _nc_cache:
        _nc_cache.append(_build())
    return _nc_cache[0]


def _prep(x, centroids):
    x = np.ascontiguousarray(np.asarray(x), dtype=np.float32)
    c = np.ascontiguousarray(np.asarray(centroids), dtype=np.float32)
    ct2 = np.ascontiguousarray((2.0 * c).reshape(K, DC, 128).transpose(2, 1, 0))
    csqr = np.ascontiguousarray(
        -(c * c).sum(-1, dtype=np.float32).reshape(1, K)
    )
    in_maps = []
    for i in range(NCORES):
        sh = x[i * NSH:(i + 1) * NSH]
        # [t, n, dc, dlow] -> [t, dlow, dc, n]
        xt = np.ascontiguousarray(
            sh.reshape(T, 128, DC, 128).transpose(0, 3, 2, 1)
        )
        in_maps.append({"xt": xt, "ct2": ct2, "csqr": csqr})
    return in_maps


def kernel(x, centroids):
    from concourse import bass_utils

    nc = _get_nc()
    in_maps = _prep(x, centroids)
    res = bass_utils.run_bass_kernel_spmd(nc, in_maps, core_ids=list(range(NCORES)))
    return np.concatenate([res.results[i]["out"] for i in range(NCORES)])

